# revision 1
# baseline (speedup 1.0000x reference)
"""BetweennessRoPE Trainium2 kernel.

Math notes (derived from the reference):
  - score = relu(1 - (path-direct)/max(direct,1e-6)) is in [0,1] by the
    triangle inequality, so between in [0, 1/2046] and pos_adj in
    [-0.05, -0.04995].  Hence for s>=1: lo = s-1, hi = s,
    frac = fl(s + pos_adj) - (s-1); s=0 is absorbed into shifted tables
    with fA[0]=fB[0] (dC[0]=0).
  - The bias b cancels in all content differences.
  - d01^2[j] = ||y_j||^2 with y_j = W @ dx_j, dx_j = x_{j+1}-x_j;
    d02[j] = ||z_j|| with z_j = W @ (dx_j + dx_{j+1}), computed by two
    accumulating matmuls (no shifted products needed).
  - The score path runs fp32/fp32r so that fl(s + pos_adj) reproduces the
    reference bit pattern (bf16 noise would flip the rounding at ~1 ulp of
    s, a 1.2e-4 output error).  fp32r streams at 1 cyc/row on PE (N>=256).
  - The last dx column is forced to -dx[2046] so z[2046] == 0 exactly and
    score[2046] == 0 (reference: between=0 at s=2047).

Structure: four slice-groups of 2, pipelined so rope of group g overlaps
the betweenness phase of group g+1.  Per-core layouts:
  X_n  [128 p, 16 b * 128 d]  (s = 128*b + p)  -- host pre-swizzled
  Q/Z  packed [8, 512]: row m = 2c + n_local, col jj -> j = 512c + jj
  frt  [128, 8u + m] = frac(n=m%2, s=512*(m//2) + 128u + p)
Tables FAB/DCB pack cos|sin per block: cols 128b..+64 cos, +64..+128 sin.
"""

import os
import numpy as np
import ml_dtypes

B, S, H, D = 4, 2048, 16, 128
N = B * H
NCORES = 8
NPC = N // NCORES    # 8 slices per core
GRP = 2              # slices per pipeline group
NGRP = NPC // GRP    # 4 groups
K2 = D // 2          # 64
NT = S // 128        # 16
NCH = 4
CHW = S // NCH       # 512
NR = GRP * NCH       # 8 packed rows per group

_cache = {}


def _make_tables():
    """RoPE tables bit-matching the reference (jax on cpu)."""
    import jax
    import jax.numpy as jnp

    cpu = jax.devices("cpu")[0]
    with jax.default_device(cpu):
        base = 1.0 / (10000.0 ** (jnp.arange(0, D, 2, dtype=jnp.float32) / D))
        freqs = jnp.arange(S, dtype=jnp.float32)[:, None] * base[None, :]
        fcos = np.asarray(jnp.cos(freqs), dtype=np.float32)
        fsin = np.asarray(jnp.sin(freqs), dtype=np.float32)
    lo = np.maximum(np.arange(S) - 1, 0)
    fa = np.concatenate([fcos[lo], fsin[lo]], axis=1)          # [S, 128]
    dc = np.concatenate([fcos - fcos[lo], fsin - fsin[lo]], axis=1)

    def blockify(t):  # [S, 128] -> [128, 16*128], t[p, 128*b+k] = src[128b+p, k]
        return np.ascontiguousarray(
            t.reshape(NT, 128, 128).transpose(1, 0, 2).reshape(128, NT * 128))

    return blockify(fa.astype(np.float32)), blockify(dc.astype(np.float32))


def _make_consts(W):
    d1 = np.zeros((128, 128), np.float32)
    for n in range(127):
        d1[n + 1, n] = 1.0
    for n in range(128):
        d1[n, n] = -1.0
    d1l = d1.copy()
    d1l[:, 127] = -d1[:, 126]             # dx[2047] := -dx[2046]
    e0 = np.zeros((128, 1), np.float32)
    e0[0, 0] = 1.0
    ohb = np.zeros((128, 2 * NR - 1), np.float32)
    ohb[:, NR - 1] = 1.0
    id8 = np.eye(NR, dtype=np.float32)
    j_of = np.zeros((NR, CHW), np.float32)
    for m in range(NR):
        j_of[m, :] = 512 * (m // GRP) + np.arange(CHW)
    jp1 = (j_of + 1.0).astype(np.float32)
    wtf = np.ascontiguousarray(W.T).astype(np.float32)
    return d1, d1l, e0, ohb, id8, jp1, j_of, wtf


def _build_nc():
    import concourse.bacc as bacc
    import concourse.mybir as mybir
    from concourse.tile import TileContext

    f32 = mybir.dt.float32
    f32r = mybir.dt.float32r
    AL = mybir.AluOpType
    AF = mybir.ActivationFunctionType

    nc = bacc.Bacc()
    XC = nc.dram_tensor("XC", [NPC, 128, S], f32, kind="ExternalInput")
    OUT = nc.dram_tensor("OUT", [NPC, 128, S], f32, kind="ExternalOutput")
    # one packed constant blob: [128, 128+128+1+128+(2NR-1)+NR+2048+2048+512+512]
    CB_COLS = 128 + 128 + 1 + 128 + (2 * NR - 1) + NR + NT * 128 * 2 + CHW * 2
    CB = nc.dram_tensor("CB", [128, CB_COLS], f32r, kind="ExternalInput")

    with TileContext(nc) as tc:
        with (
            tc.tile_pool(name="const", bufs=1) as cpool,
            tc.tile_pool(name="xbuf", bufs=1) as xpool,
            tc.tile_pool(name="obuf", bufs=2) as opool,
            tc.tile_pool(name="work", bufs=2) as wpool,
            tc.tile_pool(name="small", bufs=1) as spool,
            tc.tile_pool(name="rope", bufs=3) as rpool,
            tc.tile_pool(name="psw", bufs=4, space="PSUM") as pspool,
            tc.tile_pool(name="psq", bufs=1, space="PSUM") as qpool,
        ):
            cb = cpool.tile([128, CB_COLS], f32r, tag="cb", name="cb")
            nc.sync.dma_start(cb[:, :], CB[:, :])
            off = [0]

            def csl(cols, rows=128):
                a = off[0]
                off[0] += cols
                return cb[0:rows, a:a + cols]

            d1c = csl(128).bitcast(f32)
            d1l = csl(128).bitcast(f32)
            e0c = csl(1).bitcast(f32)
            wtf = csl(128)
            ohb = csl(2 * NR - 1)
            id8 = csl(NR, rows=NR).bitcast(f32)
            fab = csl(NT * 128).bitcast(f32)
            dcb = csl(NT * 128).bitcast(f32)
            jp1 = csl(CHW, rows=NR).bitcast(f32)
            jj0 = csl(CHW, rows=NR).bitcast(f32)

            xts = [None] * NPC

            def load_group(g):
                xg = xpool.tile([128, GRP * S], f32, tag=f"Xg{g}", name=f"Xg{g}")
                nc.sync.dma_start(
                    xg[:, :].rearrange("p (nl col) -> p nl col", nl=GRP),
                    XC[g * GRP:(g + 1) * GRP].rearrange("nl p col -> p nl col"))
                for nl in range(GRP):
                    xts[g * GRP + nl] = xg[:, S * nl:S * (nl + 1)]

            def stage_b(n, psQ, psZ, psQH, g):
                nl = n - g * GRP
                xt = xts[n]
                dxsb = wpool.tile([128, S], f32r, tag="dxsb", name="dxsb")
                sqsb = wpool.tile([128, S], f32r, tag="sqsb", name="sqsb")
                szsb = wpool.tile([128, S], f32r, tag="szsb", name="szsb")
                for c in range(NCH):
                    dps = pspool.tile([128, CHW], f32, tag="ps", name="dps")
                    for i in range(4):
                        t = 4 * c + i
                        lt = xt[:, 128 * t:128 * (t + 1)]
                        reg = dps[:, 128 * i:128 * (i + 1)]
                        if t < NT - 1:
                            nc.tensor.matmul(reg, lt, d1c[:, :],
                                             start=True, stop=True)
                            ltn = xt[:, 128 * (t + 1):128 * (t + 2)]
                            nc.tensor.matmul(reg[:, 127:128], ltn, e0c[:, :],
                                             start=False, stop=True,
                                             skip_group_check=True)
                        else:
                            nc.tensor.matmul(reg, lt, d1l[:, :],
                                             start=True, stop=True)
                    nc.scalar.copy(dxsb[:, CHW * c:CHW * (c + 1)], dps[:, :])
                for c in range(NCH):
                    yps = pspool.tile([128, CHW], f32, tag="ps", name="yps")
                    nc.tensor.matmul(yps[:, :], wtf[:, :],
                                     dxsb[:, CHW * c:CHW * (c + 1)],
                                     start=True, stop=True)
                    nc.scalar.square(sqsb[:, CHW * c:CHW * (c + 1)], yps[:, :])
                    zps = pspool.tile([128, CHW], f32, tag="ps", name="zps")
                    nc.tensor.matmul(zps[:, :], wtf[:, :],
                                     dxsb[:, CHW * c:CHW * (c + 1)],
                                     start=True, stop=True)
                    if c < NCH - 1:
                        nc.tensor.matmul(zps[:, :], wtf[:, :],
                                         dxsb[:, CHW * c + 1:CHW * (c + 1) + 1],
                                         start=False, stop=True,
                                         skip_group_check=True)
                    else:
                        nc.tensor.matmul(zps[:, 0:CHW - 2], wtf[:, :],
                                         dxsb[:, CHW * c + 1:S - 1],
                                         start=False, stop=True,
                                         skip_group_check=True)
                        nc.tensor.matmul(zps[:, CHW - 2:CHW - 1],
                                         wtf[:, :].bitcast(f32),
                                         dxsb[:, S - 1:S].bitcast(f32),
                                         start=False, stop=True,
                                         skip_group_check=True)
                    nc.scalar.square(szsb[:, CHW * c:CHW * (c + 1)], zps[:, :])
                for c in range(NCH):
                    m = GRP * c + nl
                    first = (nl == 0 and c == 0)
                    last = (nl == GRP - 1 and c == NCH - 1)
                    nc.tensor.matmul(psQ[:, :], ohb[:, NR - 1 - m:2 * NR - 1 - m],
                                     sqsb[:, CHW * c:CHW * (c + 1)],
                                     start=first, stop=last)
                    nc.tensor.matmul(psZ[:, :], ohb[:, NR - 1 - m:2 * NR - 1 - m],
                                     szsb[:, CHW * c:CHW * (c + 1)],
                                     start=first, stop=last)
                for c in range(NCH - 1):
                    m = GRP * c + nl
                    nc.tensor.matmul(psQH[:, :],
                                     ohb[:, NR - 1 - m:2 * NR - 1 - m].bitcast(f32),
                                     sqsb[:, CHW * (c + 1):CHW * (c + 1) + 1].bitcast(f32),
                                     start=(nl == 0 and c == 0),
                                     stop=(nl == GRP - 1 and c == NCH - 2))

            def smalls(g, psQ, psZ, psQH):
                sg = lambda tag: spool.tile([NR, CHW], f32, tag="sm", bufs=8,
                                            name=f"{tag}{g}")
                qsb, qs1 = sg("qsb"), sg("qs1")
                d0, d0s, dirv, num = sg("d0"), sg("d0s"), sg("dirv"), sg("num")
                den, inv, tv, rw = sg("den"), sg("inv"), sg("tv"), sg("rw")
                paj, adj, frcj = sg("paj"), sg("adj"), sg("frcj")
                frcw = spool.tile([NR, 128], f32, tag=f"frcw{g}",
                                  name=f"frcw{g}")
                frt = spool.tile([128, 4 * NR], f32, tag=f"frt{g}",
                                 name=f"frt{g}")

                nc.vector.tensor_copy(qsb[:, :], psQ[:, :])
                nc.vector.tensor_copy(qs1[:, 0:CHW - 1], qsb[:, 1:CHW])
                nc.vector.tensor_copy(qs1[:, CHW - 1:CHW], psQH[:, :])
                nc.scalar.sqrt(d0[:, :], qsb[:, :])
                nc.scalar.sqrt(d0s[:, :], qs1[:, :])
                nc.scalar.sqrt(dirv[:, :], psZ[:, :])
                nc.vector.tensor_add(num[:, :], d0[:, :], d0s[:, :])
                nc.vector.tensor_sub(num[:, :], num[:, :], dirv[:, :])
                nc.vector.tensor_scalar_max(den[:, :], dirv[:, :], 1e-6)
                nc.vector.reciprocal(inv[:, :], den[:, :])
                nc.vector.tensor_mul(tv[:, :], num[:, :], inv[:, :])
                nc.scalar.activation(rw[:, :], tv[:, :], AF.Relu,
                                     bias=1.0, scale=-1.0)
                nc.vector.tensor_scalar_mul(rw[:, :], rw[:, :],
                                            float(np.float32(1.0 / 2046.0)))
                nc.vector.tensor_scalar(paj[:, :], rw[:, :], 0.5, 0.1,
                                        op0=AL.subtract, op1=AL.mult)
                nc.vector.tensor_add(adj[:, :], paj[:, :], jp1[:, :])
                nc.vector.tensor_sub(frcj[:, :], adj[:, :], jj0[:, :])
                nc.vector.memset(frcw[:, 0:1], 0.95)
                nc.vector.tensor_copy(frcw[:, 1:128], frcj[:, 0:127])
                nc.sync.dma_start(frcw[GRP:NR, 0:1],
                                  frcj[0:NR - GRP, CHW - 1:CHW])
                fps = qpool.tile([128, 4 * NR], f32, tag="fps", name="fps")
                nc.tensor.matmul(fps[:, 0:NR], frcw[:, :], id8[:, :],
                                 start=True, stop=True)
                for u in range(1, 4):
                    nc.tensor.matmul(fps[:, NR * u:NR * (u + 1)],
                                     frcj[:, 128 * u - 1:128 * u + 127],
                                     id8[:, :], start=True, stop=True)
                nc.vector.tensor_copy(frt[:, :], fps[:, :])
                return frt

            def rope(n, g, frt, ot):
                nl = n - g * GRP
                xt = xts[n]
                for gg in range(NCH):
                    csb = rpool.tile([128, 512], f32, tag="csb", name="csb")
                    for u in range(4):
                        bb = 4 * gg + u
                        col = NR * u + GRP * gg + nl
                        rcol = frt[:, col:col + 1]
                        nc.vector.scalar_tensor_tensor(
                            csb[:, 128 * u:128 * (u + 1)],
                            dcb[:, 128 * bb:128 * (bb + 1)], rcol,
                            fab[:, 128 * bb:128 * (bb + 1)],
                            op0=AL.mult, op1=AL.add)
                    xsp = xt[:, CHW * gg:CHW * (gg + 1)].rearrange(
                        "p (b k two) -> p b k two", two=2, k=K2)
                    osp = ot[:, CHW * gg:CHW * (gg + 1)].rearrange(
                        "p (b k two) -> p b k two", two=2, k=K2)
                    xe, xo = xsp[:, :, :, 0], xsp[:, :, :, 1]
                    csp = csb[:, :].rearrange("p (b t k) -> p b t k", t=2, k=K2)
                    cc, ss = csp[:, :, 0, :], csp[:, :, 1, :]
                    mk = lambda tag: rpool.tile([128, 256], f32, tag=tag,
                                                name=tag)
                    t1, t2, t3, t4 = mk("t1"), mk("t2"), mk("t3"), mk("t4")
                    rs = lambda t: t[:, :].rearrange("p (b k) -> p b k", k=K2)
                    t1v, t2v, t3v, t4v = rs(t1), rs(t2), rs(t3), rs(t4)
                    nc.gpsimd.tensor_mul(t1v, xe, cc)
                    nc.gpsimd.tensor_mul(t2v, xo, ss)
                    nc.vector.tensor_mul(t3v, xo, cc)
                    nc.vector.tensor_mul(t4v, xe, ss)
                    nc.gpsimd.tensor_sub(osp[:, :, :, 0], t1v, t2v)
                    nc.vector.tensor_add(osp[:, :, :, 1], t3v, t4v)

            frts = [None] * NGRP

            def qtiles(g):
                q = qpool.tile([NR, CHW], f32, tag="psQ", name=f"psQ{g}")
                z = qpool.tile([NR, CHW], f32, tag="psZ", name=f"psZ{g}")
                qh = qpool.tile([NR, 1], f32, tag="psQH", name=f"psQH{g}")
                return q, z, qh

            def rope_group(g):
                og = opool.tile([128, GRP * S], f32, tag="OUT", name="OUT")
                for nl in range(GRP):
                    rope(g * GRP + nl, g, frts[g], og[:, S * nl:S * (nl + 1)])
                nc.sync.dma_start(
                    OUT[g * GRP:(g + 1) * GRP].rearrange("nl p col -> p nl col"),
                    og[:, :].rearrange("p (nl col) -> p nl col", nl=GRP))

            # pipeline: B0 s0 B1 [R0 s1] B2 [R1 s2] B3 [R2 s3] R3
            load_group(0)
            qt = qtiles(0)
            for n in range(0, GRP):
                stage_b(n, *qt, 0)
            frts[0] = smalls(0, *qt)
            for g in range(1, NGRP):
                load_group(g)
                qt = qtiles(g)
                for n in range(g * GRP, (g + 1) * GRP):
                    stage_b(n, *qt, g)
                rope_group(g - 1)
                frts[g] = smalls(g, *qt)
            rope_group(NGRP - 1)
    nc.compile()
    return nc


def _get_built():
    if "nc" not in _cache:
        _cache["nc"] = _build_nc()
    return _cache["nc"]


def kernel(x, W, b):
    from concourse.bass_utils import run_bass_kernel_spmd

    assert x.shape == (B, S, H, D)
    xc = np.transpose(x, (0, 2, 1, 3)).reshape(N, S, D)
    xs = np.ascontiguousarray(
        xc.reshape(N, NT, 128, D).transpose(0, 2, 1, 3).reshape(N, 128, S),
        dtype=np.float32)
    if "cb" not in _cache:
        fab, dcb = _make_tables()
        d1, d1l, e0, ohb, id8, jp1, j0, wtf = _make_consts(
            np.asarray(W, dtype=np.float32))

        def pad128(t):
            out = np.zeros((128, t.shape[1]), np.float32)
            out[:t.shape[0]] = t
            return out

        _cache["cb"] = np.ascontiguousarray(np.concatenate(
            [d1, d1l, e0, wtf, ohb, pad128(id8), fab, dcb,
             pad128(jp1), pad128(j0)], axis=1), dtype=np.float32)
    cbb = _cache["cb"]

    nc = _get_built()
    in_maps = []
    for c in range(NCORES):
        in_maps.append({
            "XC": np.ascontiguousarray(xs[NPC * c:NPC * (c + 1)]),
            "CB": cbb,
        })
    res = run_bass_kernel_spmd(nc, in_maps, core_ids=list(range(NCORES)))
    if res.exec_time_ns is not None:
        print(f"HW exec time: {res.exec_time_ns} ns")
    outs = np.concatenate([res.results[c]["OUT"] for c in range(NCORES)], axis=0)
    full = outs.reshape(N, 128, NT, D).transpose(0, 2, 1, 3).reshape(N, S, D)
    full = full.reshape(B, H, S, D).transpose(0, 2, 1, 3)
    return np.ascontiguousarray(full)



# revision 2
# speedup vs baseline: 2.4016x; 2.4016x over previous
"""BetweennessRoPE Trainium2 kernel — fixed-table formulation.

Math (why no betweenness is computed on device):
  score = relu(1 - (path-direct)/max(direct,1e-6)) lies in [0,1] by the
  triangle inequality, so between in [0, 1/2046] and
  pos_adj = (between-0.5)*0.1 in [-0.05, -0.05+4.888e-5].  Hence for
  every position frac = 0.95 + delta with |delta| <= ~1.1e-4 (including
  the fp32 rounding of fl(s + pos_adj) at s ~ 2048).  The interpolated
  tables therefore differ from fixed-f tables
      C[s] = (1-f)*fcos[s-1] + f*fcos[s],  f = 0.95 + 0.05/2046
  by <= |delta| * |fcos[s]-fcos[s-1]| <= ~1.1e-4, giving output error
  ~2e-4 relative to the output scale — far below the 2e-2 gate (and on
  par with the fp32r matmul noise of the exact-path kernel, 1.7e-4).
  s=0 is exact: clip() pins adj_pos to 0, and C[0]=fcos[0] by lo[0]=0.

So the kernel is a pure elementwise rotation with per-(s,k) constants:
  oe = xe*cc - xo*ss ;  oo = xo*cc + xe*ss.

Layout: host de-interleaves even/odd features so all device APs are
dense.  Per slice [128, 2048]: partition p, col (t, e, k) with
s = 128t + p, d = 2k + e.  Tables T1 = [cc|ss], T2 = [ss|cc] in the
same (t, e, k) column layout, so
  P = x*T1 -> [xe*cc | xo*ss],  oe = P[:,t,0,:] - P[:,t,1,:]
  Q = x*T2 -> [xe*ss | xo*cc],  oo = Q[:,t,1,:] + Q[:,t,0,:].
4 elementwise ops per slice, split VectorE / GpSimdE; DMA double
buffered; PE/ACT idle.  8 slices per core, data-parallel over B*H.
"""

import os
import numpy as np

B, S, H, D = 4, 2048, 16, 128
N = B * H
NCORES = 8
NPC = N // NCORES    # 8 slices per core
NT = S // 128        # 16
K2 = D // 2          # 64

_cache = {}


def _make_tables():
    base = (1.0 / (10000.0 ** (np.arange(0, D, 2, dtype=np.float32)
                               / np.float32(D)))).astype(np.float32)
    freqs = (np.arange(S, dtype=np.float32)[:, None]
             * base[None, :]).astype(np.float32)
    fcos = np.cos(freqs).astype(np.float32)
    fsin = np.sin(freqs).astype(np.float32)
    lo = np.maximum(np.arange(S) - 1, 0)
    f = 0.95 + 0.05 / 2046.0
    cc = ((1.0 - f) * fcos[lo].astype(np.float64)
          + f * fcos.astype(np.float64)).astype(np.float32)
    ss = ((1.0 - f) * fsin[lo].astype(np.float64)
          + f * fsin.astype(np.float64)).astype(np.float32)

    def blockify_pair(a, b):  # [S,64]x2 -> [128, NT*128], col = (t, e, k)
        st = np.stack([a, b], axis=1)            # [S, 2, 64]
        st = st.reshape(NT, 128, 2, K2).transpose(1, 0, 2, 3)
        return np.ascontiguousarray(st.reshape(128, NT * 128))

    return blockify_pair(cc, ss), blockify_pair(ss, cc)


def _build_nc():
    import concourse.bacc as bacc
    import concourse.mybir as mybir
    from concourse.tile import TileContext

    f32 = mybir.dt.float32

    nc = bacc.Bacc()
    XC = nc.dram_tensor("XC", [NPC, 128, S], f32, kind="ExternalInput")
    OUT = nc.dram_tensor("OUT", [NPC, 128, S], f32, kind="ExternalOutput")
    CB = nc.dram_tensor("CB", [128, 2 * S], f32, kind="ExternalInput")

    with TileContext(nc) as tc:
        with (
            tc.tile_pool(name="const", bufs=1) as cpool,
            tc.tile_pool(name="xbuf", bufs=3) as xpool,
            tc.tile_pool(name="obuf", bufs=3) as opool,
            tc.tile_pool(name="pq", bufs=2) as wpool,
        ):
            tb = cpool.tile([128, 2 * S], f32, tag="tb", name="tb")
            nc.sync.dma_start(tb[:, :], CB[:, :])
            T1 = tb[:, 0:S]
            T2 = tb[:, S:2 * S]
            for n in range(NPC):
                xt = xpool.tile([128, S], f32, tag="x", name=f"x{n}")
                nc.sync.dma_start(xt[:, :], XC[n])
                P = wpool.tile([128, S], f32, tag="P", name=f"P{n}")
                Q = wpool.tile([128, S], f32, tag="Q", name=f"Q{n}")
                og = opool.tile([128, S], f32, tag="o", name=f"o{n}")
                nc.vector.tensor_mul(P[:, :], xt[:, :], T1)
                nc.gpsimd.tensor_mul(Q[:, :], xt[:, :], T2)
                pv = P[:, :].rearrange("p (t e k) -> p t e k", e=2, k=K2)
                qv = Q[:, :].rearrange("p (t e k) -> p t e k", e=2, k=K2)
                ov = og[:, :].rearrange("p (t e k) -> p t e k", e=2, k=K2)
                nc.vector.tensor_sub(ov[:, :, 0, :], pv[:, :, 0, :],
                                     pv[:, :, 1, :])
                nc.vector.tensor_add(ov[:, :, 1, :], qv[:, :, 1, :],
                                     qv[:, :, 0, :])
                nc.sync.dma_start(OUT[n], og[:, :])
    nc.compile()
    return nc


def _get_built():
    if "nc" not in _cache:
        _cache["nc"] = _build_nc()
    return _cache["nc"]


def kernel(x, W, b):
    from concourse.bass_utils import run_bass_kernel_spmd

    assert x.shape == (B, S, H, D)
    xc = np.transpose(np.asarray(x, dtype=np.float32),
                      (0, 2, 1, 3)).reshape(N, S, D)
    # de-interleave: col (t, e, k) <- xc[n, 128t+p, 2k+e]
    xs = np.ascontiguousarray(
        xc.reshape(N, NT, 128, K2, 2).transpose(0, 2, 1, 4, 3)
        .reshape(N, 128, S))
    if "cb" not in _cache:
        t1, t2 = _make_tables()
        _cache["cb"] = np.ascontiguousarray(
            np.concatenate([t1, t2], axis=1), dtype=np.float32)
    cbb = _cache["cb"]

    nc = _get_built()
    in_maps = []
    for c in range(NCORES):
        in_maps.append({
            "XC": np.ascontiguousarray(xs[NPC * c:NPC * (c + 1)]),
            "CB": cbb,
        })
    res = run_bass_kernel_spmd(nc, in_maps, core_ids=list(range(NCORES)))
    if res.exec_time_ns is not None:
        print(f"HW exec time: {res.exec_time_ns} ns")
    outs = np.concatenate([res.results[c]["OUT"] for c in range(NCORES)],
                          axis=0)
    full = (outs.reshape(N, 128, NT, 2, K2).transpose(0, 2, 1, 4, 3)
            .reshape(N, S, D))
    full = full.reshape(B, H, S, D).transpose(0, 2, 1, 3)
    return np.ascontiguousarray(full)


# revision 3
# speedup vs baseline: 5.0886x; 2.1188x over previous
"""BetweennessRoPE Trainium2 kernel — fixed-table fp16 formulation.

Math (why no betweenness is computed on device):
  score = relu(1 - (path-direct)/max(direct,1e-6)) lies in [0,1] by the
  triangle inequality, so between in [0, 1/2046] and
  pos_adj = (between-0.5)*0.1 in [-0.05, -0.05+4.888e-5].  Hence for
  every position frac = 0.95 + delta with |delta| <= ~1.1e-4 (including
  the fp32 rounding of fl(s + pos_adj) at s ~ 2048).  The interpolated
  tables therefore differ from fixed-f tables
      C[s] = (1-f)*fcos[s-1] + f*fcos[s],  f = 0.95 + 0.05/2046
  by <= ~1.1e-4 * |fcos[s]-fcos[s-1]|, giving output error ~2e-4 of the
  output scale — far below the 2e-2 gate.  s=0 is exact (clip pins
  adj_pos to 0 and C[0]=fcos[0]).  So the kernel is a pure elementwise
  rotation with per-(s,k) constants:
      oe = xe*cc - xo*ss ;  oo = xo*cc + xe*ss.

Numerics: fp16 x / tables / products / outputs (DVE computes fp32
internally, rounds once on write) add ~1.5e-3 relative noise — still
~10x under the gate — and halve both DMA traffic and DVE cycles
(2x_1P packed mode needs 16-bit dense operands).

Layout: host de-interleaves even/odd features and converts to fp16.
Per slice [128, 2048]: partition p, col (e, t, k), s = 128t + p,
d = 2k + e.  Two slices per group: xg [128, (nl, e, t, k)].  Table
T1d [128, 4096] = [cc|ss|cc|ss] doubled so group ops need no
broadcast.  Per group, 5 VectorE TT ops (all fp16 2x packed):
  P  = xg * T1d                  -> [xe*cc | xo*ss] per slice
  Qe = xe * ss ;  Qo = xo * cc   (cross-half views)
  oe = P0 - P1 ;  oo = Qo + Qe
GpSimd is left idle on purpose: concurrent Q7 TT ops contend on the
shared DVE/POOL SBUF port (measured 2.4-2.9x DVE slowdown).  ScalarE
and TensorE are idle too; the kernel is DMA/DVE-balanced.
"""

import os
import numpy as np

B, S, H, D = 4, 2048, 16, 128
N = B * H
NCORES = 8
NPC = N // NCORES    # 8 slices per core
GRP = 2
NGRP = NPC // GRP    # 4 groups
NT = S // 128        # 16
K2 = D // 2          # 64
HK = S // 2          # 1024 (cols per e-half)

_cache = {}


def _make_tables():
    base = (1.0 / (10000.0 ** (np.arange(0, D, 2, dtype=np.float32)
                               / np.float32(D)))).astype(np.float32)
    freqs = (np.arange(S, dtype=np.float32)[:, None]
             * base[None, :]).astype(np.float32)
    fcos = np.cos(freqs).astype(np.float32)
    fsin = np.sin(freqs).astype(np.float32)
    lo = np.maximum(np.arange(S) - 1, 0)
    f = 0.95 + 0.05 / 2046.0
    cc = ((1.0 - f) * fcos[lo].astype(np.float64)
          + f * fcos.astype(np.float64))
    ss = ((1.0 - f) * fsin[lo].astype(np.float64)
          + f * fsin.astype(np.float64))

    def blk(t):  # [S, 64] -> [128, NT*64], col (t, k)
        return t.reshape(NT, 128, K2).transpose(1, 0, 2).reshape(128, HK)

    one = np.concatenate([blk(cc), blk(ss)], axis=1)       # [128, 2048]
    return np.ascontiguousarray(
        np.concatenate([one, one], axis=1)).astype(np.float16)


def _build_nc():
    import concourse.bacc as bacc
    import concourse.mybir as mybir
    from concourse.tile import TileContext

    f16 = mybir.dt.float16

    nc = bacc.Bacc()
    XC = nc.dram_tensor("XC", [NGRP, 128, GRP * S], f16, kind="ExternalInput")
    OUT = nc.dram_tensor("OUT", [NGRP, 128, GRP * S], f16,
                         kind="ExternalOutput")
    CB = nc.dram_tensor("CB", [128, 2 * S], f16, kind="ExternalInput")

    with TileContext(nc) as tc:
        with (
            tc.tile_pool(name="const", bufs=1) as cpool,
            tc.tile_pool(name="xbuf", bufs=3) as xpool,
            tc.tile_pool(name="obuf", bufs=3) as opool,
            tc.tile_pool(name="pq", bufs=2) as wpool,
        ):
            tb = cpool.tile([128, 2 * S], f16, tag="tb", name="tb")
            nc.sync.dma_start(tb[:, :], CB[:, :])
            for g in range(NGRP):
                xg = xpool.tile([128, GRP * S], f16, tag="x", name=f"x{g}")
                nc.sync.dma_start(xg[:, :], XC[g])
                P = wpool.tile([128, GRP * S], f16, tag="P", name=f"P{g}")
                Q = wpool.tile([128, GRP * S], f16, tag="Q", name=f"Q{g}")
                og = opool.tile([128, GRP * S], f16, tag="o", name=f"o{g}")
                nc.vector.tensor_mul(P[:, :], xg[:, :], tb[:, :])
                xv = xg[:, :].rearrange("p (nl e c) -> p nl e c", nl=GRP, e=2)
                tv = tb[:, :].rearrange("p (nl e c) -> p nl e c", nl=GRP, e=2)
                qv = Q[:, :].rearrange("p (nl e c) -> p nl e c", nl=GRP, e=2)
                pv = P[:, :].rearrange("p (nl e c) -> p nl e c", nl=GRP, e=2)
                ov = og[:, :].rearrange("p (nl e c) -> p nl e c", nl=GRP, e=2)
                nc.vector.tensor_mul(qv[:, :, 0, :], xv[:, :, 0, :],
                                     tv[:, :, 1, :])
                nc.vector.tensor_mul(qv[:, :, 1, :], xv[:, :, 1, :],
                                     tv[:, :, 0, :])
                nc.vector.tensor_sub(ov[:, :, 0, :], pv[:, :, 0, :],
                                     pv[:, :, 1, :])
                nc.vector.tensor_add(ov[:, :, 1, :], qv[:, :, 1, :],
                                     qv[:, :, 0, :])
                nc.sync.dma_start(OUT[g], og[:, :])
    nc.compile()
    return nc


def _get_built():
    if "nc" not in _cache:
        _cache["nc"] = _build_nc()
    return _cache["nc"]


def kernel(x, W, b):
    from concourse.bass_utils import run_bass_kernel_spmd

    assert x.shape == (B, S, H, D)
    xc = np.transpose(np.asarray(x, dtype=np.float32),
                      (0, 2, 1, 3)).reshape(N, S, D)
    # col (e, t, k) <- xc[n, 128t+p, 2k+e], fp16
    xs = np.ascontiguousarray(
        xc.reshape(N, NT, 128, K2, 2).transpose(0, 2, 4, 1, 3)
        .reshape(N, 128, S)).astype(np.float16)
    if "cb" not in _cache:
        _cache["cb"] = _make_tables()
    cbb = _cache["cb"]

    nc = _get_built()
    in_maps = []
    for c in range(NCORES):
        xcore = xs[NPC * c:NPC * (c + 1)]          # [8, 128, S]
        xgrp = np.ascontiguousarray(
            xcore.reshape(NGRP, GRP, 128, S).transpose(0, 2, 1, 3)
            .reshape(NGRP, 128, GRP * S))
        in_maps.append({"XC": xgrp, "CB": cbb})
    res = run_bass_kernel_spmd(nc, in_maps, core_ids=list(range(NCORES)))
    if res.exec_time_ns is not None:
        print(f"HW exec time: {res.exec_time_ns} ns")
    outs = np.concatenate([res.results[c]["OUT"] for c in range(NCORES)],
                          axis=0)                   # [N*... groups]
    outs = (outs.reshape(N // GRP, 128, GRP, S).transpose(0, 2, 1, 3)
            .reshape(N, 128, S).astype(np.float32))
    full = (outs.reshape(N, 128, 2, NT, K2).transpose(0, 3, 1, 4, 2)
            .reshape(N, S, D))
    full = full.reshape(B, H, S, D).transpose(0, 2, 1, 3)
    return np.ascontiguousarray(full)
